# revision 1
# baseline (speedup 1.0000x reference)
"""AlternateGCN on 8 TRN2 NeuronCores.

Strategy (edge/data parallel, adapted from the sharding hint to this HW):
  - Edges sharded by DESTINATION-node owner core (8 node shards of 6272),
    sorted by destination. Segment-sum runs ON DEVICE as dense PE matmuls:
    per 128-edge tile a selection matrix St[e, n] = c_e * (col_e == n) is
    built with one vector op and Y[window] += St^T @ Mt accumulates in PSUM.
    No indexed writes; aggregation is shard-local => no collectives needed.
  - GCN two-sided normalization folded into per-edge coefficients
    (c_e = w_e * dinv[row_e]) and per-window output scales (dinv[col]).
  - Host does ONLY index preprocessing and data movement (shard/sort/pad,
    fancy-index row gathers between pipeline stages); all f32 arithmetic
    (degree reduce, rsqrt, matmuls, ELU/ReLU MLPs) runs on device.
  - 4 NEFF stages: N1 degrees->dinv; N2 layer-1 conv -> h~2 shard;
    N3 layer-2 conv -> u,v decoder tables; N4 edge MLP decoder -> out.

Self-contained; hardcodes problem shapes (N=50000, E=800000, D=128, H=256).
"""
import sys, time
sys.path.insert(0, "/opt/trn_rl_repo")
import numpy as np
import jax
from jax.sharding import Mesh, PartitionSpec
from jax.experimental.shard_map import shard_map

import concourse.bacc as bacc
import concourse.bass as bass
import concourse.mybir as mybir
import concourse.tile as tile
from concourse import bass2jax
from concourse.bass2jax import _bass_exec_p, install_neuronx_cc_hook

P = 128
NCORES = 8
AF = mybir.ActivationFunctionType
ALU = mybir.AluOpType
DT = mybir.dt
VERBOSE = bool(int(__import__("os").environ.get("KERNEL_VERBOSE", "0")))


def _log(msg):
    if VERBOSE:
        print(f"[kernel {time.strftime('%H:%M:%S')}] {msg}", flush=True)


# ---------------------------------------------------------------- runner ----
class SpmdRunner:
    def __init__(self, nc, n_cores=NCORES, donate=True):
        install_neuronx_cc_hook()
        self.nc, self.n_cores = nc, n_cores
        self.donate = donate
        pname = nc.partition_id_tensor.name if nc.partition_id_tensor else None
        in_names, out_names, out_avals, zero_outs = [], [], [], []
        for alloc in nc.m.functions[0].allocations:
            if not isinstance(alloc, mybir.MemoryLocationSet):
                continue
            name = alloc.memorylocations[0].name
            if alloc.kind == "ExternalInput":
                if name != pname:
                    in_names.append(name)
            elif alloc.kind == "ExternalOutput":
                out_names.append(name)
                shape = tuple(alloc.tensor_shape)
                dtype = DT.np(alloc.dtype)
                out_avals.append(jax.core.ShapedArray(shape, dtype))
                zero_outs.append(np.zeros(shape, dtype))
        self.in_names, self.out_names = in_names, out_names
        self.out_avals, self.zero_outs = out_avals, zero_outs
        self.n_params, self.n_outs = len(in_names), len(out_avals)
        all_in = list(in_names) + list(out_names)
        if pname is not None:
            all_in.append(pname)

        def _body(*args):
            operands = list(args)
            if pname is not None:
                operands.append(bass2jax.partition_id_tensor())
            return tuple(_bass_exec_p.bind(
                *operands, out_avals=tuple(out_avals), in_names=tuple(all_in),
                out_names=tuple(out_names), lowering_input_output_aliases=(),
                sim_require_finite=True, sim_require_nnan=True, nc=nc))

        devices = jax.devices()[:n_cores]
        mesh = Mesh(np.asarray(devices), ("core",))
        donate = tuple(range(self.n_params, self.n_params + self.n_outs))             if self.donate else ()
        self.fn = jax.jit(
            shard_map(_body, mesh=mesh,
                      in_specs=(PartitionSpec("core"),) * (self.n_params + self.n_outs),
                      out_specs=(PartitionSpec("core"),) * len(out_names)),
            donate_argnums=donate, keep_unused=True)
        self.dev_inputs = None
        self.dev_zeros = None

    def stage(self, in_maps):
        concat = [np.concatenate([np.ascontiguousarray(in_maps[c][n])
                                  for c in range(self.n_cores)], axis=0)
                  for n in self.in_names]
        self.dev_inputs = [jax.device_put(a) for a in concat]
        jax.block_until_ready(self.dev_inputs)

    def run(self):
        if self.donate:
            zeros = [np.zeros((self.n_cores * z.shape[0], *z.shape[1:]), z.dtype)
                     for z in self.zero_outs]
        else:
            if self.dev_zeros is None:
                self.dev_zeros = [jax.device_put(
                    np.zeros((self.n_cores * z.shape[0], *z.shape[1:]), z.dtype))
                    for z in self.zero_outs]
                jax.block_until_ready(self.dev_zeros)
            zeros = self.dev_zeros
        out = self.fn(*self.dev_inputs, *zeros)
        jax.block_until_ready(out)
        return out

    def results(self, out_arrs):
        return [
            {n: np.asarray(out_arrs[i]).reshape(self.n_cores, *self.out_avals[i].shape)[c]
             for i, n in enumerate(self.out_names)}
            for c in range(self.n_cores)]


# ------------------------------------------------------------- geometry ----
class Cfg:
    def __init__(self, N=50000, D=128, H=256):
        self.N, self.D, self.H = N, D, H
        self.NPAD = ((N + NCORES * P - 1) // (NCORES * P)) * (NCORES * P)
        self.SHARD = self.NPAD // NCORES
        self.WPC = self.SHARD // P           # windows per core
        self.WGL = self.NPAD // P            # global windows


# ------------------------------------------------------- device helpers ----
def _rsqrt_masked(nc, out_sb, deg_sb, pool, shape):
    """out = 1/sqrt(deg) where deg>0 else 0 (no Rsqrt, no inf*0)."""
    m = pool.tile(shape, DT.float32, tag="rq_m")
    t = pool.tile(shape, DT.float32, tag="rq_t")
    nc.vector.tensor_scalar(out=m[:], in0=deg_sb, scalar1=0.0, scalar2=None,
                            op0=ALU.is_gt)
    nc.scalar.activation(out=t[:], in_=deg_sb, func=AF.Sqrt)
    nc.vector.tensor_scalar(out=t[:], in0=t[:], scalar1=1.0, scalar2=None,
                            op0=ALU.add)
    nc.vector.tensor_tensor(out=t[:], in0=t[:], in1=m[:], op=ALU.subtract)
    nc.vector.reciprocal(out=out_sb, in_=t[:])
    nc.vector.tensor_tensor(out=out_sb, in0=out_sb, in1=m[:], op=ALU.mult)


def _elu(nc, out_sb, z_sb, pool, shape):
    """out = elu(z) = relu(z) + exp(min(z,0)) - 1."""
    r = pool.tile(shape, DT.float32, tag="elu_r")
    e = pool.tile(shape, DT.float32, tag="elu_e")
    nc.scalar.activation(out=r[:], in_=z_sb, func=AF.Relu)
    nc.vector.tensor_scalar(out=e[:], in0=z_sb, scalar1=0.0, scalar2=None,
                            op0=ALU.min)
    nc.scalar.activation(out=e[:], in_=e[:], func=AF.Exp)
    nc.vector.tensor_tensor(out=out_sb, in0=r[:], in1=e[:], op=ALU.add)
    nc.vector.tensor_scalar(out=out_sb, in0=out_sb, scalar1=-1.0, scalar2=None,
                            op0=ALU.add)


# ------------------------------------------------------------------ N1 -----
def build_n1(cfg, K1, repeat=1):
    """deg1 (padded-slot reduce over all E edges, replicated) -> dinv1;
    deg2 host counts -> dinv2. Outputs in [p, t] = node t*128+p layout."""
    nc = bacc.Bacc("TRN2", target_bir_lowering=False)
    W = cfg.WGL
    wpad = nc.dram_tensor("wpad", [P, W * K1], DT.float32, kind="ExternalInput")
    d2c = nc.dram_tensor("d2c", [P, W], DT.float32, kind="ExternalInput")
    dinv1 = nc.dram_tensor("dinv1", [P, W], DT.float32, kind="ExternalOutput")
    dinv2 = nc.dram_tensor("dinv2", [P, W], DT.float32, kind="ExternalOutput")
    with tile.TileContext(nc) as tc:
        with tc.tile_pool(name="p", bufs=1) as pool:
            for _ in range(repeat):
                wt = pool.tile([P, W, K1], DT.float32, tag="wt")
                nc.sync.dma_start(out=wt[:].rearrange("p a b -> p (a b)"),
                                  in_=wpad[:, :])
                deg = pool.tile([P, W], DT.float32, tag="deg")
                nc.vector.tensor_reduce(out=deg[:, :, None], in_=wt[:],
                                        axis=mybir.AxisListType.X, op=ALU.add)
                o1 = pool.tile([P, W], DT.float32, tag="o1")
                _rsqrt_masked(nc, o1[:], deg[:], pool, [P, W])
                nc.sync.dma_start(out=dinv1[:, :], in_=o1[:])
                d2 = pool.tile([P, W], DT.float32, tag="d2")
                nc.sync.dma_start(out=d2[:], in_=d2c[:, :])
                o2 = pool.tile([P, W], DT.float32, tag="o2")
                _rsqrt_masked(nc, o2[:], d2[:], pool, [P, W])
                nc.sync.dma_start(out=dinv2[:, :], in_=o2[:])
    nc.finalize()
    return nc


# ----------------------------------------------------------- N2 and N3 -----
def build_conv(cfg, TPW, layer, repeat=1):
    """Conv layer as matmul-scatter + node pipeline.

    layer=1: Y1 -> x1=elu(dinv1*Y1 @ W_in + b_in) -> h2=x1@W_out
             -> h~2 = dinv2*h2 -> out shard [SHARD, D].
    layer=2: Y2 -> x2=elu(dinv2*Y2 + b_out) -> u = x2@W1a + b1, v = x2@W1b
             -> out shards u, v [SHARD, D].
    dv1/dv2 inputs are PER-CORE pre-sliced [P, WPC] (this core's windows).
    """
    nc = bacc.Bacc("TRN2", target_bir_lowering=False)
    D, H, WPC = cfg.D, cfg.H, cfg.WPC
    T = WPC * TPW
    m1 = nc.dram_tensor("m1", [P, T * D], DT.float32, kind="ExternalInput")
    ct = nc.dram_tensor("ct", [P, T], DT.float32, kind="ExternalInput")
    wv = nc.dram_tensor("wv", [P, T], DT.float32, kind="ExternalInput")
    c0 = nc.dram_tensor("c0", [P, T], DT.float32, kind="ExternalInput")
    iot = nc.dram_tensor("iot", [P, P], DT.float32, kind="ExternalInput")
    idt = nc.dram_tensor("idt", [P, P], DT.float32, kind="ExternalInput")
    if layer == 1:
        w_in = nc.dram_tensor("w_in", [D, H], DT.float32, kind="ExternalInput")
        b_in = nc.dram_tensor("b_in", [P, 2], DT.float32, kind="ExternalInput")
        w_out = nc.dram_tensor("w_out", [H, D], DT.float32, kind="ExternalInput")
        dv1 = nc.dram_tensor("dv1", [P, WPC], DT.float32, kind="ExternalInput")
        dv2 = nc.dram_tensor("dv2", [P, WPC], DT.float32, kind="ExternalInput")
        h2s = nc.dram_tensor("h2s", [cfg.SHARD, D], DT.float32, kind="ExternalOutput")
    else:
        w1 = nc.dram_tensor("w1", [2 * D, D], DT.float32, kind="ExternalInput")
        b1v = nc.dram_tensor("b1v", [P, 1], DT.float32, kind="ExternalInput")
        b_out = nc.dram_tensor("b_out", [P, 1], DT.float32, kind="ExternalInput")
        dv2 = nc.dram_tensor("dv2", [P, WPC], DT.float32, kind="ExternalInput")
        us = nc.dram_tensor("us", [cfg.SHARD, D], DT.float32, kind="ExternalOutput")
        vs = nc.dram_tensor("vs", [cfg.SHARD, D], DT.float32, kind="ExternalOutput")
    with tile.TileContext(nc) as tc:
        with (tc.tile_pool(name="const", bufs=1) as cst,
              tc.tile_pool(name="stream", bufs=1) as stp,
              tc.tile_pool(name="mt", bufs=3) as mtp,
              tc.tile_pool(name="st", bufs=6) as sstp,
              tc.tile_pool(name="ypsum", bufs=2, space="PSUM") as yp,
              tc.tile_pool(name="tpsum", bufs=1, space="PSUM") as tp,
              tc.tile_pool(name="fl", bufs=3) as fl):
            iot_sb = cst.tile([P, P], DT.float32)
            idt_sb = cst.tile([P, P], DT.float32)
            nc.sync.dma_start(out=iot_sb[:], in_=iot[:, :])
            nc.sync.dma_start(out=idt_sb[:], in_=idt[:, :])
            if layer == 1:
                w_in_sb = cst.tile([P, H], DT.float32)
                nc.sync.dma_start(out=w_in_sb[:], in_=w_in[:, :])
                wo_a = cst.tile([P, D], DT.float32)
                wo_b = cst.tile([P, D], DT.float32)
                nc.sync.dma_start(out=wo_a[:], in_=w_out[0:P, :])
                nc.sync.dma_start(out=wo_b[:], in_=w_out[P:2 * P, :])
                b_in_sb = cst.tile([P, 2], DT.float32)
                nc.sync.dma_start(out=b_in_sb[:], in_=b_in[:, :])
                dv1_sb = cst.tile([P, WPC], DT.float32)
                nc.sync.dma_start(out=dv1_sb[:], in_=dv1[:, :])
            else:
                w1a = cst.tile([P, D], DT.float32)
                w1b = cst.tile([P, D], DT.float32)
                nc.sync.dma_start(out=w1a[:], in_=w1[0:P, :])
                nc.sync.dma_start(out=w1b[:], in_=w1[P:2 * P, :])
                b1_sb = cst.tile([P, 1], DT.float32)
                nc.sync.dma_start(out=b1_sb[:], in_=b1v[:, :])
                bo_sb = cst.tile([P, 1], DT.float32)
                nc.sync.dma_start(out=bo_sb[:], in_=b_out[:, :])
            dv2_sb = cst.tile([P, WPC], DT.float32)
            nc.sync.dma_start(out=dv2_sb[:], in_=dv2[:, :])
            ct_sb = stp.tile([P, T], DT.float32)
            cc_sb = stp.tile([P, T], DT.float32)
            c0_sb = stp.tile([P, T], DT.float32)
            nc.sync.dma_start(out=ct_sb[:], in_=ct[:, :])
            nc.sync.dma_start(out=cc_sb[:], in_=wv[:, :])
            nc.sync.dma_start(out=c0_sb[:], in_=c0[:, :])
            nc.vector.tensor_tensor(out=cc_sb[:], in0=cc_sb[:], in1=c0_sb[:],
                                    op=ALU.mult)

            for w in [w for _ in range(repeat) for w in range(WPC)]:
                y = yp.tile([P, D], DT.float32, tag="y")
                mw = mtp.tile([P, TPW * D], DT.float32)
                nc.sync.dma_start(out=mw[:],
                                  in_=m1[:, w * TPW * D:(w + 1) * TPW * D])
                for s in range(TPW):
                    t = w * TPW + s
                    st = sstp.tile([P, P], DT.float32)
                    nc.vector.tensor_scalar(
                        out=st[:], in0=iot_sb[:], scalar1=ct_sb[:, t:t + 1],
                        scalar2=cc_sb[:, t:t + 1], op0=ALU.is_equal, op1=ALU.mult)
                    nc.tensor.matmul(out=y[:], lhsT=st[:],
                                     rhs=mw[:, s * D:(s + 1) * D],
                                     start=(s == 0), stop=(s == TPW - 1))
                if layer == 1:
                    ys = fl.tile([P, D], DT.float32, tag="ys")
                    nc.vector.tensor_scalar(out=ys[:], in0=y[:],
                                            scalar1=dv1_sb[:, w:w + 1],
                                            scalar2=None, op0=ALU.mult)
                    yst_p = tp.tile([P, P], DT.float32, tag="yst")
                    nc.tensor.transpose(out=yst_p[:], in_=ys[:], identity=idt_sb[:])
                    yst = fl.tile([P, P], DT.float32, tag="yst_sb")
                    nc.vector.tensor_copy(out=yst[:], in_=yst_p[:])
                    zs, rs, es = [], [], []
                    for hh in range(2):
                        xp = tp.tile([P, P], DT.float32, tag=f"x1p{hh}")
                        nc.tensor.matmul(out=xp[:],
                                         lhsT=w_in_sb[:, hh * P:(hh + 1) * P],
                                         rhs=yst[:], start=True, stop=True)
                        z = fl.tile([P, P], DT.float32, tag=f"z{hh}")
                        nc.vector.tensor_scalar(out=z[:], in0=xp[:],
                                                scalar1=b_in_sb[:, hh:hh + 1],
                                                scalar2=None, op0=ALU.add)
                        zs.append(z)
                    # grouped ACT funcs: Relu,Relu then Exp,Exp (1 table swap)
                    for hh in range(2):
                        r = fl.tile([P, P], DT.float32, tag=f"r{hh}")
                        nc.scalar.activation(out=r[:], in_=zs[hh][:], func=AF.Relu)
                        rs.append(r)
                    for hh in range(2):
                        e = fl.tile([P, P], DT.float32, tag=f"e{hh}")
                        nc.vector.tensor_scalar(out=e[:], in0=zs[hh][:],
                                                scalar1=0.0, scalar2=None,
                                                op0=ALU.min)
                        nc.scalar.activation(out=e[:], in_=e[:], func=AF.Exp)
                        es.append(e)
                    x1t = []
                    for hh in range(2):
                        xe = fl.tile([P, P], DT.float32, tag=f"xe{hh}")
                        nc.vector.tensor_tensor(out=xe[:], in0=rs[hh][:],
                                                in1=es[hh][:], op=ALU.add)
                        nc.vector.tensor_scalar(out=xe[:], in0=xe[:],
                                                scalar1=-1.0, scalar2=None,
                                                op0=ALU.add)
                        x1t.append(xe)
                    h2p = tp.tile([P, P], DT.float32, tag="h2p")
                    nc.tensor.matmul(out=h2p[:], lhsT=wo_a[:], rhs=x1t[0][:],
                                     start=True, stop=False)
                    nc.tensor.matmul(out=h2p[:], lhsT=wo_b[:], rhs=x1t[1][:],
                                     start=False, stop=True)
                    h2sb = fl.tile([P, P], DT.float32, tag="h2sb")
                    nc.vector.tensor_copy(out=h2sb[:], in_=h2p[:])
                    h2t_p = tp.tile([P, P], DT.float32, tag="h2tp")
                    nc.tensor.transpose(out=h2t_p[:], in_=h2sb[:],
                                        identity=idt_sb[:])
                    hs = fl.tile([P, D], DT.float32, tag="hs")
                    nc.vector.tensor_scalar(out=hs[:], in0=h2t_p[:],
                                            scalar1=dv2_sb[:, w:w + 1],
                                            scalar2=None, op0=ALU.mult)
                    nc.sync.dma_start(out=h2s[w * P:(w + 1) * P, :], in_=hs[:])
                else:
                    ys = fl.tile([P, D], DT.float32, tag="ys")
                    nc.vector.tensor_scalar(out=ys[:], in0=y[:],
                                            scalar1=dv2_sb[:, w:w + 1],
                                            scalar2=None, op0=ALU.mult)
                    yst_p = tp.tile([P, P], DT.float32, tag="yst")
                    nc.tensor.transpose(out=yst_p[:], in_=ys[:], identity=idt_sb[:])
                    z = fl.tile([P, P], DT.float32, tag="z")
                    nc.vector.tensor_scalar(out=z[:], in0=yst_p[:],
                                            scalar1=bo_sb[:, 0:1],
                                            scalar2=None, op0=ALU.add)
                    x2t = fl.tile([P, P], DT.float32, tag="x2t")
                    _elu(nc, x2t[:], z[:], fl, [P, P])
                    up = tp.tile([P, P], DT.float32, tag="up")
                    nc.tensor.matmul(out=up[:], lhsT=w1a[:], rhs=x2t[:],
                                     start=True, stop=True)
                    ut = fl.tile([P, P], DT.float32, tag="ut")
                    nc.vector.tensor_scalar(out=ut[:], in0=up[:],
                                            scalar1=b1_sb[:, 0:1], scalar2=None,
                                            op0=ALU.add)
                    vp = tp.tile([P, P], DT.float32, tag="vp")
                    nc.tensor.matmul(out=vp[:], lhsT=w1b[:], rhs=x2t[:],
                                     start=True, stop=True)
                    vt = fl.tile([P, P], DT.float32, tag="vt")
                    nc.vector.tensor_copy(out=vt[:], in_=vp[:])
                    u_p = tp.tile([P, P], DT.float32, tag="u_p")
                    nc.tensor.transpose(out=u_p[:], in_=ut[:], identity=idt_sb[:])
                    u_sb = fl.tile([P, D], DT.float32, tag="u_sb")
                    nc.vector.tensor_copy(out=u_sb[:], in_=u_p[:])
                    nc.sync.dma_start(out=us[w * P:(w + 1) * P, :], in_=u_sb[:])
                    v_p = tp.tile([P, P], DT.float32, tag="v_p")
                    nc.tensor.transpose(out=v_p[:], in_=vt[:], identity=idt_sb[:])
                    v_sb = fl.tile([P, D], DT.float32, tag="v_sb")
                    nc.vector.tensor_copy(out=v_sb[:], in_=v_p[:])
                    nc.sync.dma_start(out=vs[w * P:(w + 1) * P, :], in_=v_sb[:])
    nc.finalize()
    return nc


# ------------------------------------------------------------------ N4 -----
def build_n4(cfg, TPW4, repeat=1):
    """Decoder: H1 = relu(u[src] + v[dst]); out = relu(H1@W2+b2)@W3+b3.
    u[src] host-streamed TRANSPOSED ([f, e]); v[dst] reconstructed on device
    via StT matmul against the local v-shard window (dst sorted)."""
    nc = bacc.Bacc("TRN2", target_bir_lowering=False)
    D, WPC = cfg.D, cfg.WPC
    CPW = TPW4 // 4                     # 512-edge chunks per window
    T = WPC * TPW4
    E_SLOTS = T * P
    gut = nc.dram_tensor("gut", [P, E_SLOTS], DT.float32, kind="ExternalInput")
    cv = nc.dram_tensor("cv", [1, E_SLOTS], DT.float32, kind="ExternalInput")
    vsh = nc.dram_tensor("vsh", [cfg.SHARD, D], DT.float32, kind="ExternalInput")
    w2 = nc.dram_tensor("w2", [D, D], DT.float32, kind="ExternalInput")
    b2v = nc.dram_tensor("b2v", [P, 1], DT.float32, kind="ExternalInput")
    w3 = nc.dram_tensor("w3", [D, 1], DT.float32, kind="ExternalInput")
    b3v = nc.dram_tensor("b3v", [1, 1], DT.float32, kind="ExternalInput")
    pio = nc.dram_tensor("pio", [P, 1], DT.float32, kind="ExternalInput")
    one = nc.dram_tensor("one", [1, P], DT.float32, kind="ExternalInput")
    oute = nc.dram_tensor("oute", [1, E_SLOTS], DT.float32, kind="ExternalOutput")
    CH = 512
    with tile.TileContext(nc) as tc:
        with (tc.tile_pool(name="const", bufs=1) as cst,
              tc.tile_pool(name="vw", bufs=3) as vwp,
              tc.tile_pool(name="cvp", bufs=3) as cvp,
              tc.tile_pool(name="gu", bufs=3) as gup,
              tc.tile_pool(name="stt", bufs=4) as sttp,
              tc.tile_pool(name="bps", bufs=2, space="PSUM") as bps,
              tc.tile_pool(name="vps", bufs=2, space="PSUM") as vps,
              tc.tile_pool(name="h2ps", bufs=2, space="PSUM") as h2ps,
              tc.tile_pool(name="ops", bufs=1, space="PSUM") as ops,
              tc.tile_pool(name="sb", bufs=4) as sbp):
            w2_sb = cst.tile([P, D], DT.float32)
            nc.sync.dma_start(out=w2_sb[:], in_=w2[:, :])
            w3_sb = cst.tile([P, 1], DT.float32)
            nc.sync.dma_start(out=w3_sb[:], in_=w3[:, :])
            b2_sb = cst.tile([P, 1], DT.float32)
            nc.sync.dma_start(out=b2_sb[:], in_=b2v[:, :])
            b3_sb = cst.tile([1, 1], DT.float32)
            nc.sync.dma_start(out=b3_sb[:], in_=b3v[:, :])
            pio_sb = cst.tile([P, 1], DT.float32)
            nc.sync.dma_start(out=pio_sb[:], in_=pio[:, :])
            one_sb = cst.tile([1, P], DT.float32)
            nc.sync.dma_start(out=one_sb[:], in_=one[:, :])
            for w in [w for _ in range(repeat) for w in range(WPC)]:
                vwin = vwp.tile([P, D], DT.float32)
                nc.sync.dma_start(out=vwin[:], in_=vsh[w * P:(w + 1) * P, :])
                cvw = cvp.tile([1, TPW4 * P], DT.float32)
                nc.sync.dma_start(out=cvw[:],
                                  in_=cv[:, w * TPW4 * P:(w + 1) * TPW4 * P])
                guw = gup.tile([P, TPW4 * P], DT.float32)
                nc.sync.dma_start(out=guw[:],
                                  in_=gut[:, w * TPW4 * P:(w + 1) * TPW4 * P])
                for cc in range(CPW):
                    ch = w * CPW + cc
                    sl = slice(ch * CH, (ch + 1) * CH)
                    lsl = slice(cc * CH, (cc + 1) * CH)
                    cb = bps.tile([P, CH], DT.float32, tag="cb")
                    nc.tensor.matmul(out=cb[:], lhsT=one_sb[:],
                                     rhs=cvw[:, lsl], start=True, stop=True)
                    stt = sttp.tile([P, CH], DT.float32)
                    nc.vector.tensor_scalar(out=stt[:], in0=cb[:],
                                            scalar1=pio_sb[:, 0:1], scalar2=None,
                                            op0=ALU.is_equal)
                    vtt = vps.tile([P, CH], DT.float32, tag="vtt")
                    nc.tensor.matmul(out=vtt[:], lhsT=vwin[:], rhs=stt[:],
                                     start=True, stop=True)
                    h1 = sbp.tile([P, CH], DT.float32, tag="h1")
                    nc.vector.tensor_tensor(out=h1[:], in0=vtt[:],
                                            in1=guw[:, lsl], op=ALU.add)
                    h1r = sbp.tile([P, CH], DT.float32, tag="h1r")
                    nc.scalar.activation(out=h1r[:], in_=h1[:], func=AF.Relu)
                    h2 = h2ps.tile([P, CH], DT.float32, tag="h2")
                    nc.tensor.matmul(out=h2[:], lhsT=w2_sb[:], rhs=h1r[:],
                                     start=True, stop=True)
                    h2r = sbp.tile([P, CH], DT.float32, tag="h2r")
                    nc.scalar.activation(out=h2r[:], in_=h2[:], func=AF.Relu,
                                         bias=b2_sb[:, 0:1])
                    op = ops.tile([1, CH], DT.float32, tag="op")
                    nc.tensor.matmul(out=op[:], lhsT=w3_sb[:], rhs=h2r[:],
                                     start=True, stop=True)
                    ot = sbp.tile([1, CH], DT.float32, tag="ot")
                    nc.vector.tensor_scalar(out=ot[:], in0=op[:],
                                            scalar1=b3_sb[:, 0:1], scalar2=None,
                                            op0=ALU.add)
                    nc.sync.dma_start(out=oute[:, sl], in_=ot[:])
    nc.finalize()
    return nc


# ------------------------------------------------------------ host prep ----
def _shard_edges(cfg, ei, budget_tpw):
    """Shard edges by col-owner core, sort by col, place into fixed
    per-window tile budgets. Returns per-core dict with padded slot arrays.

    Slot layout per core: windows w=0..WPC-1, each owns budget_tpw tiles of
    128 slots; edge i in slot (t, p) lives at stream position t*128+p.
    """
    row, col = np.asarray(ei[0]), np.asarray(ei[1])
    owner = col // cfg.SHARD
    cores = []
    maxpw = 0
    for c in range(NCORES):
        sel = np.where(owner == c)[0]
        o = np.argsort(col[sel], kind="stable")
        sel = sel[o]
        colc = col[sel]
        win = (colc - c * cfg.SHARD) // P
        cnt = np.bincount(win, minlength=cfg.WPC)
        maxpw = max(maxpw, int(cnt.max()))
        cores.append((sel, colc, win, cnt))
    tpw = max(budget_tpw, (maxpw + P - 1) // P)
    tpw = ((tpw + 3) // 4) * 4          # chunk-of-512 alignment for N4 reuse
    S = cfg.WPC * tpw * P               # slots per core
    out = []
    for c in range(NCORES):
        sel, colc, win, cnt = cores[c]
        # slot index per edge: window base + position within window
        offs = np.zeros(cfg.WPC + 1, np.int64)
        np.cumsum(cnt, out=offs[1:])
        pos_in_win = np.arange(len(sel)) - offs[win]
        slot = win.astype(np.int64) * (tpw * P) + pos_in_win
        rows_s = np.zeros(S, np.int64)          # pad -> row 0
        cols_rel = np.zeros(S, np.float32)      # pad -> col_rel 0
        valid = np.zeros(S, bool)
        rows_s[slot] = row[sel]
        cols_rel[slot] = (colc - (colc // P) * P).astype(np.float32)
        valid[slot] = True
        out.append(dict(sel=sel, slot=slot, rows=rows_s, cols_rel=cols_rel,
                        valid=valid, tpw=tpw, S=S))
    return out, tpw


def _wrap_stream(vec, S):
    """[S] -> [128, S/128] with slot (t, p) at [p, t]: tiles on free axis."""
    return np.ascontiguousarray(vec.reshape(S // P, P).T)


def _feat_stream(tbl, rows, S, D):
    """[S] row ids -> [128, S/128 * D] tile stream: tile t partition p
    holds tbl[rows[t*128+p]]."""
    g = tbl[rows.reshape(S // P, P)]          # [T, 128, D]
    return np.ascontiguousarray(g.transpose(1, 0, 2).reshape(P, (S // P) * D))


def _col_layout(vec_full, cfg):
    """[NPAD] -> [128, WGL]: node t*128+p at [p, t]."""
    return np.ascontiguousarray(vec_full.reshape(cfg.WGL, P).T)


# --------------------------------------------------------------- kernel ----
_TIMES = {}
_DBG = {}


def kernel(node_ids, edge_index, neighbour_edge_index, edge_attr,
           emb, W_in, b_in, W_out, b_out, W1, b1, W2, b2, W3, b3):
    cfg = Cfg()
    D, H = cfg.D, cfg.H
    t_all = time.time()
    node_ids = np.asarray(node_ids)
    emb = np.asarray(emb, np.float32)
    edge_attr = np.asarray(edge_attr, np.float32)
    x = emb[node_ids]                               # [N, D]
    x_pad = np.zeros((cfg.NPAD, D), np.float32)
    x_pad[:cfg.N] = x

    # ---- host index prep -------------------------------------------------
    e1, tpw1 = _shard_edges(cfg, np.asarray(edge_index), 18)
    e2, tpw2 = _shard_edges(cfg, np.asarray(neighbour_edge_index), 18)
    # decoder reuses e1's ordering but its own budget (chunk-of-512 aligned)
    tpw4 = tpw1

    # N1 host data: padded multiplicity layout for deg1 (all E edges,
    # replicated to every core); host-side bincount (index data) for deg2.
    col1 = np.asarray(edge_index)[1]
    col2 = np.asarray(neighbour_edge_index)[1]
    o = np.argsort(col1, kind="stable")
    cs, ws_ = col1[o], edge_attr[o]
    cnt1 = np.bincount(cs, minlength=cfg.NPAD)
    offs = np.zeros(cfg.NPAD + 1, np.int64)
    np.cumsum(cnt1, out=offs[1:])
    slot1 = np.arange(len(cs)) - offs[cs]
    K1 = int(cnt1.max())
    wpad = np.zeros((cfg.NPAD, K1), np.float32)
    wpad[cs, slot1] = ws_
    wpad_l = np.ascontiguousarray(
        wpad.reshape(cfg.WGL, P, K1).transpose(1, 0, 2).reshape(P, cfg.WGL * K1))
    d2c_l = _col_layout(np.bincount(col2, minlength=cfg.NPAD).astype(np.float32), cfg)

    iot = np.tile(np.arange(P, dtype=np.float32)[None, :], (P, 1))
    idt = np.eye(P, dtype=np.float32)
    pio = np.arange(P, dtype=np.float32)[:, None]
    one = np.ones((1, P), np.float32)

    # ---- N1 --------------------------------------------------------------
    _log(f"N1 build (K1={K1})")
    n1 = build_n1(cfg, K1)
    r1 = SpmdRunner(n1, donate=False)
    r1.stage([{"wpad": wpad_l, "d2c": d2c_l}] * NCORES)
    t0 = time.time(); out1 = r1.run(); _TIMES["n1"] = time.time() - t0
    res1 = r1.results(out1)
    dinv1_l = res1[0]["dinv1"]                       # [128, WGL]
    dinv2_l = res1[0]["dinv2"]
    dinv1 = dinv1_l.T.reshape(-1)                    # [NPAD] node-order
    dinv2 = dinv2_l.T.reshape(-1)
    _DBG["dinv1"], _DBG["dinv2"] = dinv1, dinv2
    _log("N1 done")

    # ---- N2 --------------------------------------------------------------
    n2 = build_conv(cfg, tpw1, layer=1)
    r2 = SpmdRunner(n2, donate=False)
    maps2 = []
    W_in_np = np.asarray(W_in, np.float32)
    b_in_np = np.asarray(b_in, np.float32)
    W_out_np = np.asarray(W_out, np.float32)
    row1 = np.asarray(edge_index)[0]
    for c in range(NCORES):
        ec = e1[c]
        S = ec["S"]
        wv_s = np.zeros(S, np.float32)
        wv_s[ec["slot"]] = edge_attr[ec["sel"]]
        c0_s = dinv1[ec["rows"]] * ec["valid"]       # host gather (movement)
        maps2.append({
            "m1": _feat_stream(x_pad, ec["rows"], S, D),
            "ct": _wrap_stream(ec["cols_rel"], S),
            "wv": _wrap_stream(wv_s, S),
            "c0": _wrap_stream(c0_s.astype(np.float32), S),
            "w_in": W_in_np, "b_in": b_in_np.reshape(2, P).T.copy(),
            "w_out": W_out_np,
            "dv1": dinv1_l[:, c * cfg.WPC:(c + 1) * cfg.WPC],
            "dv2": dinv2_l[:, c * cfg.WPC:(c + 1) * cfg.WPC],
            "iot": iot, "idt": idt,
        })
    r2.stage(maps2)
    _log("N2 staged")
    t0 = time.time(); out2 = r2.run(); _TIMES["n2"] = time.time() - t0
    res2 = r2.results(out2)
    h2full = np.concatenate([res2[c]["h2s"] for c in range(NCORES)], axis=0)
    _DBG["h2full"] = h2full
    _log("N2 done")

    # ---- N3 --------------------------------------------------------------
    n3 = build_conv(cfg, tpw2, layer=2)
    r3 = SpmdRunner(n3, donate=False)
    maps3 = []
    W1_np = np.asarray(W1, np.float32)
    row2 = np.asarray(neighbour_edge_index)[0]
    for c in range(NCORES):
        ec = e2[c]
        S = ec["S"]
        wv_s = ec["valid"].astype(np.float32)
        c0_s = ec["valid"]  # row-side dinv2 already folded into h~2
        maps3.append({
            "m1": _feat_stream(h2full, ec["rows"], S, D),
            "ct": _wrap_stream(ec["cols_rel"], S),
            "wv": _wrap_stream(wv_s, S),
            "c0": _wrap_stream(c0_s.astype(np.float32), S),
            "w1": W1_np, "b1v": np.asarray(b1, np.float32)[:, None],
            "b_out": np.asarray(b_out, np.float32)[:, None],
            "dv2": dinv2_l[:, c * cfg.WPC:(c + 1) * cfg.WPC],
            "iot": iot, "idt": idt,
        })
    r3.stage(maps3)
    _log("N3 staged")
    t0 = time.time(); out3 = r3.run(); _TIMES["n3"] = time.time() - t0
    res3 = r3.results(out3)
    ufull = np.concatenate([res3[c]["us"] for c in range(NCORES)], axis=0)
    _DBG["ufull"] = ufull
    _DBG["vfull"] = np.concatenate([res3[c]["vs"] for c in range(NCORES)], axis=0)
    _log("N3 done")

    # ---- N4 --------------------------------------------------------------
    n4 = build_n4(cfg, tpw4)
    r4 = SpmdRunner(n4, donate=False)
    maps4 = []
    for c in range(NCORES):
        ec = e1[c]
        S = ec["S"]
        maps4.append({
            "gut": np.ascontiguousarray(ufull[ec["rows"]].T),
            "cv": ec["cols_rel"][None, :],
            "vsh": res3[c]["vs"],
            "w2": np.asarray(W2, np.float32),
            "b2v": np.asarray(b2, np.float32)[:, None],
            "w3": np.asarray(W3, np.float32),
            "b3v": np.asarray(b3, np.float32)[None, :],
            "pio": pio, "one": one,
        })
    r4.stage(maps4)
    _log("N4 staged")
    t0 = time.time(); out4 = r4.run(); _TIMES["n4"] = time.time() - t0
    res4 = r4.results(out4)
    _log("N4 done")

    # ---- unshard ---------------------------------------------------------
    E = np.asarray(edge_index).shape[1]
    result = np.zeros(E, np.float32)
    for c in range(NCORES):
        oc = res4[c]["oute"][0]
        result[e1[c]["sel"]] = oc[e1[c]["slot"]]
    _TIMES["total_wall"] = time.time() - t_all
    _DBG["runners"] = {"n1": r1, "n2": r2, "n3": r3, "n4": r4}
    _DBG["meta"] = {"K1": K1, "tpw1": tpw1, "tpw2": tpw2, "tpw4": tpw4}
    return result

